# revision 47
# baseline (speedup 1.0000x reference)
"""Trainium2 Bass kernel for a 3-sublayer decoder block (nn_DecoderLayer).

Reference computation (B=2, S=2048, D=1024, H=16, DK=64, FF=4096, fp32):
  sa = causal_mha(x, x)          ; x1 = seqnorm(sa + x)
  ca = mha(x1, enc)              ; x2 = seqnorm(ca + x1)
  ffn = relu(x2 @ W1 + b1) @ W2 + b2 ; out = seqnorm(ffn + x2)
seqnorm normalizes over the SEQUENCE dim and divides by the unbiased VARIANCE
(reference quirk); attention has no output projection.

Sharding (8 cores): heads split 2-per-core for both attentions (each core owns
a 128-wide channel slice of the attention outputs end-to-end, so seqnorm over
S stays local for x1/x2 stats).  The FFN is TOKEN-parallel: each core computes
the full 4096-wide hidden for its 512 tokens, obtained via tiny AllToAlls of
the raw x2 (no AllGather+ReduceScatter pair).  Collectives are overlapped with
compute by gathering RAW (pre-norm) activations while attention still runs and
broadcasting only the per-channel norm stats (r = VARF/var, c = -mean*r) at
sublayer boundaries; consumers fold the normalization in on the fly (for q2 it
is folded into per-batch-scaled weight copies + a constant bias term).

Schedule (the engines execute their queues IN ORDER, so emission order is the
scheduling instrument):
 - everything pipelines by BATCH: attn1(b0) -> [x1 AllGathers + stats for b0
   land while attn1(b1) runs] -> attn2(b0) -> attn2(b1) -> FFN, with the
   encoder k/v projections emitted as PE filler right after each batch's
   self-attention;
 - the attention chunk loop is software-pipelined with a lag of one
   (scores(c+1) before exp(c)/PV(c)) so the PE never head-blocks on the ACT
   engine, which paces the chunk steady state at ~1.15us/chunk;
 - in cross-attention the NEXT tile's q2 projection is emitted between the
   current tile's chunks and its normalize chain, so the DVE->gpsimd->DVE
   softmax-normalize latency stays off the PE's critical path;
 - softmax 1/denominator uses reciprocal_approx_fast (plain InstReciprocal
   costs ~4.8us/call on the DVE and was the top serializer);
 - FFN: h-phase per batch (batch-0's h matmuls cover batch-1's last AllToAll
   + stats latency), y-phase in two output-channel halves with cross-core
   seqnorm moments as per-half (mean, E[x^2]) 8KB AllReduces, so half 0's
   stats collective + finalize + output DMA overlap half 1's matmuls.

All matmuls run in bf16 (fp32 PSUM accumulation); the 2e-2 rel-err gate leaves
~3x margin at the observed ~6.1e-3.  Activations live transposed on-chip
([d, s]); attention computes E^T = exp(K @ Q^T) tiles ([sk, sq]) with the two
heads' score matmuls row-tiled to concurrent PE halves; softmax denominators
come from an appended ones-column on V.  FFN weights are streamed from HBM
(never resident): W1 per 128-wide ff-slice during h, W2 half-rows during y
into 8 concurrently-live PSUM accumulators.
"""

import os
import sys

import numpy as np

for _p in ("/opt/trn_rl_repo", "/root/.axon_site/_ro/trn_rl_repo"):
    if _p not in sys.path and os.path.isdir(_p):
        sys.path.append(_p)

import concourse.bass as bass
import concourse.mybir as mybir
import concourse.tile as tile
from concourse import bacc
from concourse.bass import ts
from concourse.bass_utils import run_bass_kernel_spmd

F32 = mybir.dt.float32
BF16 = mybir.dt.bfloat16
AF = mybir.ActivationFunctionType
ALU = mybir.AluOpType

B, S, D, H = 2, 2048, 1024, 16
DK = D // H            # 64
FF = 4 * D             # 4096
NCORES = 8
HL = H // NCORES       # 2 heads per core
DL = DK * HL           # 128 channels per core
KC = D // 128          # 8 contraction chunks of the model dim
FC = FF // 128         # 32 ff chunks
ST = S // 512          # 4 sequence tiles of 512
SC = S // 128          # 16 sequence chunks of 128
TOK = B * S // NCORES  # 512 tokens per core in the FFN phase
SCALE = 1.0 / np.sqrt(DK)
VARF = (S - 1) / S     # unbiased-variance factor applied to 1/var_pop

RG = [[0, 1, 2, 3, 4, 5, 6, 7]]

_CACHED_NC = None


def _build():
    nc = bacc.Bacc("TRN2", target_bir_lowering=False, debug=False,
                   num_devices=NCORES)

    # ---- per-core external inputs ----
    xTd = nc.dram_tensor("xTd", [128, B, KC, S], BF16, kind="ExternalInput")
    encd = nc.dram_tensor("encd", [128, B, KC, S], BF16, kind="ExternalInput")
    resd = nc.dram_tensor("resd", [128, B, S], F32, kind="ExternalInput")
    wq1d = nc.dram_tensor("wq1d", [128, KC, DL], BF16, kind="ExternalInput")
    wk1d = nc.dram_tensor("wk1d", [128, KC, DL], BF16, kind="ExternalInput")
    wv1d = nc.dram_tensor("wv1d", [128, KC, DL], BF16, kind="ExternalInput")
    wq2d = nc.dram_tensor("wq2d", [128, KC, DL], BF16, kind="ExternalInput")
    wk2d = nc.dram_tensor("wk2d", [128, KC, DL], BF16, kind="ExternalInput")
    wv2d = nc.dram_tensor("wv2d", [128, KC, DL], BF16, kind="ExternalInput")
    w1d = nc.dram_tensor("w1d", [128, FC, KC, 128], BF16, kind="ExternalInput")
    w2d = nc.dram_tensor("w2d", [128, FC, D], BF16, kind="ExternalInput")
    bqkd = nc.dram_tensor("bqkd", [128, 4], F32, kind="ExternalInput")
    bvd = nc.dram_tensor("bvd", [64, 2 * HL], F32, kind="ExternalInput")
    b1d = nc.dram_tensor("b1d", [128, FC], F32, kind="ExternalInput")
    b2d = nc.dram_tensor("b2d", [128, KC], F32, kind="ExternalInput")

    outT = nc.dram_tensor("outT", [128, KC, TOK], F32, kind="ExternalOutput")
    DBG = bool(os.environ.get("BASSDBG"))
    if DBG:
        dbg_x1 = nc.dram_tensor("dbg_x1", [128, B, S], F32,
                                kind="ExternalOutput")
        dbg_xn = nc.dram_tensor("dbg_xn", [128, KC, TOK], BF16,
                                kind="ExternalOutput")

    def rview(t):   # [8*B*128, s] -> [p, rank, b, s]
        return t[:].rearrange("(r b p) s -> p r b s", r=NCORES, b=B)

    def bview(t):   # [B*128, s] -> [p, b, s]
        return t[:].rearrange("(b p) s -> p b s", p=128)

    with tile.TileContext(nc) as tc:
        import contextlib
        ctx = contextlib.ExitStack()
        with ctx:
            sb = ctx.enter_context(tc.tile_pool(name="sb", bufs=1))
            dram = ctx.enter_context(tc.tile_pool(name="dr", bufs=1,
                                                  space="DRAM"))
            ps = ctx.enter_context(tc.tile_pool(name="ps", bufs=2,
                                                space="PSUM"))

            # ---- collective bounce buffers ----
            # x1 is gathered in eight per-(tile, batch) AllGathers so each
            # batch's gather (and its seqnorm stats, riding as 2 extra bf16
            # columns on the batch's last tile) completes while the OTHER
            # batch's attention still runs: the sublayers pipeline by batch
            # with no collective on the critical path
            XB = [512, 512, 512, 514]
            x1bh = [[dram.tile([128, XB[t]], BF16, tag=f"x1b{t}{b}",
                               name=f"x1b{t}{b}") for b in range(B)]
                    for t in range(ST)]
            x1fh = [[dram.tile([NCORES * 128, XB[t]], BF16,
                               tag=f"x1f{t}{b}", name=f"x1f{t}{b}",
                               addr_space="Shared") for b in range(B)]
                    for t in range(ST)]
            a2ain = [dram.tile([NCORES * 128, 64], BF16, tag=f"a2i{g}",
                               name=f"a2i{g}") for g in range(2 * ST)]
            a2aout = [dram.tile([NCORES * 128, 64], BF16, tag=f"a2o{g}",
                                name=f"a2o{g}") for g in range(2 * ST)]
            st2b = [dram.tile([128, 2], F32, tag=f"st2b{b}",
                              name=f"st2b{b}") for b in range(B)]
            st2f = [dram.tile([NCORES * 128, 2], F32, tag=f"st2f{b}",
                              name=f"st2f{b}", addr_space="Shared")
                    for b in range(B)]
            st3b = dram.tile([128, B * KC * 6], F32, tag="st3b",
                             name="st3b")
            st3f = dram.tile([NCORES * 128, B * KC * 6], F32, tag="st3f",
                             name="st3f", addr_space="Shared")

            # ---- small persistent tiles ----
            # (FFN biases + v-bias go on the scalar queue: they aren't
            # needed for hundreds of us and must not delay the first xs
            # activation loads on the sync queue)
            bqk_sb = sb.tile([128, 4], F32, tag="bias", bufs=1)
            nc.sync.dma_start(out=bqk_sb, in_=bqkd[:])
            bv_sb = sb.tile([64, 2 * HL], F32, tag="bias2", bufs=1)
            nc.scalar.dma_start(out=bv_sb, in_=bvd[:])
            b1_sb = sb.tile([128, FC], F32, tag="bias3", bufs=1)
            nc.scalar.dma_start(out=b1_sb, in_=b1d[:])
            b2_sb = sb.tile([128, KC], F32, tag="bias4", bufs=1)
            nc.scalar.dma_start(out=b2_sb, in_=b2d[:])

            # identity for PE transposes (bf16: transpose at 1.0 cyc/row)
            id32 = sb.tile([128, 128], BF16, tag="id32", bufs=1)
            nc.vector.memset(id32, 1.0)
            nc.gpsimd.affine_select(out=id32, in_=id32,
                                    compare_op=ALU.is_equal, fill=0.0,
                                    base=0, channel_multiplier=-1,
                                    pattern=[[1, 128]])

            def load_w(dram_t, name, eng=None):
                w = sb.tile([128, KC, DL], BF16, tag="wp", bufs=6, name=name)
                (eng or nc.sync).dma_start(out=w, in_=dram_t[:])
                return w

            wq1 = load_w(wq1d, "wq1")
            wk1 = load_w(wk1d, "wk1")
            wv1 = load_w(wv1d, "wv1", eng=nc.scalar)

            kT1 = sb.tile([128, B, S], BF16, tag="kv", bufs=2, name="kT1")
            vO1 = sb.tile([128, B, SC, HL, DK + 1], BF16, tag="vo", bufs=2,
                          name="vO1")
            x1 = sb.tile([128, B, S], F32, tag="xl", bufs=1, name="x1")

            def proj128(psrc, w, bias_col, out_ap):
                """One [128, 512] projection: out = W.T @ x + bias."""
                p_ = ps.tile([128, 512], F32, tag="a", name="p_")
                for k in range(KC):
                    nc.tensor.matmul(p_, w[:, k, :], psrc[:, k, :],
                                     start=(k == 0), stop=(k == KC - 1))
                nc.vector.tensor_scalar(
                    out=out_ap, in0=p_,
                    scalar1=bqk_sb[:, bias_col:bias_col + 1],
                    scalar2=None, op0=ALU.add)

            def vproj(psrc, wv, kT, vO, b, t, qw=None, qcol=None, kw=None,
                      kcol=None, qt_list=None):
                """k/v (and optionally q) projections for one (b, t) tile;
                v is flipped back to key-major via PE transposes into vO with
                an appended ones column."""
                if qw is not None:
                    qt = sb.tile([128, 512], BF16, tag="qt", bufs=3,
                                 name="qt")
                    proj128(psrc, qw, qcol, qt[:, :])
                    qt_list.append(qt)
                if kw is not None:
                    proj128(psrc, kw, kcol, kT[:, b, ts(t, 512)])
                vt = sb.tile([128, 512], BF16, tag="vt", bufs=2, name="vt")
                p_ = ps.tile([128, 512], F32, tag="a", name="p_")
                for k in range(KC):
                    nc.tensor.matmul(p_, wv[:, k, :], psrc[:, k, :],
                                     start=(k == 0), stop=(k == KC - 1))
                nc.vector.tensor_copy(out=vt, in_=p_)
                for sc in range(4):
                    c = 4 * t + sc
                    tp = ps.tile([128, 128], BF16, tag="a", name="tp")
                    nc.tensor.transpose(tp, vt[:, ts(sc, 128)], id32)
                    for h in range(HL):
                        nc.vector.tensor_copy(out=vO[:, b, c, h, 0:DK],
                                              in_=tp[:, ts(h, DK)])
                    nc.vector.tensor_scalar(
                        out=vO[:, b, c, :, DK:DK + 1],
                        in0=tp[:, 0:HL][:, :, None],
                        scalar1=0.0, scalar2=1.0,
                        op0=ALU.mult, op1=ALU.add)

            def attn_one(b, t, qt, kT, vO, bv_off, causal, resid_cb,
                         out_cb):
                """One (b, sq-tile) of attention.  E^T chunks for both heads
                packed in one [128,1024] psum; exp on the ACT engine; PV with
                the ones-row denominator.  The chunk loop is software-
                pipelined with a lag of one: scores(c+1) is emitted BEFORE
                exp(c)/PV(c) so the in-order PE queue never head-blocks on
                the ACT engine.  Normalize/bias/residual handles both heads
                in one pass (resid_cb(h)/out_cb(h) -> [64,512] APs)."""
                nchunks = (4 * t + 4) if causal else SC
                zh = [ps.tile([128, 512], F32, tag="z", name=f"zh{h}")
                      for h in range(HL)]

                def scores(c):
                    # on causal diagonal chunks the first 128*j query columns
                    # are fully masked: skip them in the scores matmul (the
                    # affine_select below zeroes that region of et anyway)
                    q0 = 128 * (c - 4 * t) if causal and c >= 4 * t else 0
                    eps = ps.tile([128, 1024], F32, tag="e", name="eps")
                    for h in range(HL):
                        hb = h * 64
                        nc.tensor.matmul(eps[:, h * 512 + q0:(h + 1) * 512],
                                         kT[hb:hb + 64, b, ts(c, 128)],
                                         qt[hb:hb + 64, q0:512],
                                         start=True, stop=True)
                    return (c, q0, eps)

                def exp_pv(pend):
                    c, q0, eps = pend
                    et = sb.tile([128, 1024], BF16, tag="et", bufs=3,
                                 name="et")
                    j = c - 4 * t
                    if causal and c >= 4 * t and q0 >= 256:
                        # deep diagonal chunks: restrict exp to live columns
                        for h in range(HL):
                            nc.scalar.activation(
                                out=et[:, h * 512 + q0:(h + 1) * 512],
                                in_=eps[:, h * 512 + q0:(h + 1) * 512],
                                func=AF.Exp, scale=float(SCALE))
                    else:
                        nc.scalar.activation(out=et, in_=eps, func=AF.Exp,
                                             scale=float(SCALE))
                    if causal and c >= 4 * t:
                        for h in range(HL):
                            nc.gpsimd.affine_select(
                                out=et[:, h * 512 + q0:(h + 1) * 512],
                                in_=et[:, h * 512 + q0:(h + 1) * 512],
                                compare_op=ALU.is_ge, fill=0.0,
                                base=q0 - j * 128, channel_multiplier=-1,
                                pattern=[[1, 512 - q0]])
                    for h in range(HL):
                        # masked query columns are exact zeros in et: skip
                        # them in PV too (group check bypassed because the
                        # restricted last chunk stops only live columns)
                        nc.tensor.matmul(zh[h][0:DK + 1, q0:512],
                                         vO[:, b, c, h, :],
                                         et[:, h * 512 + q0:(h + 1) * 512],
                                         start=(c == 0),
                                         stop=(c == nchunks - 1),
                                         skip_group_check=True)

                pend = None
                for c in range(nchunks):
                    cur = scores(c)
                    if pend is not None:
                        exp_pv(pend)
                    pend = cur
                exp_pv(pend)

                # drain numerator+denominator of both heads out of psum fast
                # (frees the z banks for the next tile); the remaining
                # normalize chain is returned as a closure so callers can
                # emit the NEXT tile's PE-feeding work first (the gpsimd
                # broadcast hop injects latency into the in-order DVE queue)
                zn = sb.tile([DK + 1, 1024], F32, tag="zn", bufs=1,
                             name="zn")
                for h in range(HL):
                    nc.vector.tensor_copy(out=zn[:, ts(h, 512)],
                                          in_=zh[h][0:DK + 1, :])

                def finish():
                    dr = sb.tile([1, 1024], F32, tag="dr", bufs=1,
                                 name="dr")
                    nc.vector.tensor_copy(out=dr, in_=zn[DK:DK + 1, :])
                    # approx reciprocal (~18 bits): denominators are sums of
                    # exps (positive, normal range), and plain InstReciprocal
                    # costs ~4.8us per call on the DVE
                    nc.vector.reciprocal_approx_fast(out=dr, in_=dr)
                    rb = sb.tile([64, 1024], F32, tag="rb", bufs=1,
                                 name="rb")
                    nc.gpsimd.partition_broadcast(out_ap=rb, in_ap=dr)
                    t1 = sb.tile([64, 1024], F32, tag="t1", bufs=1,
                                 name="t1")
                    nc.vector.tensor_mul(t1, zn[0:DK, :], rb)
                    for h in range(HL):
                        nc.vector.scalar_tensor_tensor(
                            out=out_cb(h), in0=t1[:, ts(h, 512)],
                            scalar=bv_sb[:, bv_off + h:bv_off + h + 1],
                            op0=ALU.add, in1=resid_cb(h), op1=ALU.add)

                return finish

            # ================= sublayer 1: causal self-attention ===========
            st1 = sb.tile([128, B, ST, 6], F32, tag="st1", bufs=1,
                          name="st1")
            qts1 = []
            for b in range(B):
                for t in range(ST):
                    xs = sb.tile([128, KC, 512], BF16, tag="xs", bufs=4,
                                 name="xs")
                    # split every load across both DMA queues so each
                    # projection chain's input lands in half the time
                    nc.gpsimd.dma_start(out=xs[:, 0:4, :],
                                        in_=xTd[:, b, 0:4, ts(t, 512)])
                    nc.sync.dma_start(out=xs[:, 4:8, :],
                                      in_=xTd[:, b, 4:8, ts(t, 512)])
                    vproj(xs, wv1, kT1, vO1, b, t, qw=wq1, qcol=0,
                          kw=wk1, kcol=1, qt_list=qts1)

            wq2 = load_w(wq2d, "wq2")
            wk2 = load_w(wk2d, "wk2")
            wv2 = load_w(wv2d, "wv2", eng=nc.scalar)
            kT2 = sb.tile([128, B, S], BF16, tag="kv", bufs=2, name="kT2")
            vO2 = sb.tile([128, B, SC, HL, DK + 1], BF16, tag="vo", bufs=2,
                          name="vO2")
            rc1 = sb.tile([128, 2 * B], F32, tag="rc1", bufs=1, name="rc1")

            def attn1_tile(b, t):
                # residual staged head-major at base partition 0 (the fused
                # normalize op needs both SBUF inputs on the same partitions)
                rs_ = sb.tile([64, 2, 512], F32, tag="rs", bufs=2,
                              name="rs_")
                for h in range(HL):
                    nc.gpsimd.dma_start(
                        out=rs_[:, h, :],
                        in_=resd[h * 64:h * 64 + 64, b, ts(t, 512)])

                def res1(h):
                    return rs_[:, h, :]

                def out1(h):
                    return x1[h * 64:h * 64 + 64, b, ts(t, 512)]

                attn_one(b, t, qts1[4 * b + t], kT1, vO1, bv_off=0,
                         causal=True, resid_cb=res1, out_cb=out1)()
                stage = sb.tile([128, 512], BF16, tag="stage", bufs=2,
                                name="stage")
                nc.vector.tensor_copy(out=stage, in_=x1[:, b, ts(t, 512)])
                nc.gpsimd.dma_start(out=x1bh[t][b][0:128, 0:512],
                                    in_=stage)
                nc.vector.bn_stats(out=st1[:, b, t, :],
                                   in_=x1[:, b, ts(t, 512)])

            def stats1_close(b):
                # local stats -> (r, c); ride the last tile's AllGather,
                # then normalize the local x1 slice in place (residual for
                # sublayer 2)
                mv = sb.tile([128, 2], F32, tag="bnmv", bufs=2, name="mv")
                nc.vector.bn_aggr(out=mv, in_=st1[:, b, :, :])
                nc.vector.reciprocal(rc1[:, 2 * b:2 * b + 1], mv[:, 1:2])
                nc.vector.tensor_scalar(out=rc1[:, 2 * b:2 * b + 1],
                                        in0=rc1[:, 2 * b:2 * b + 1],
                                        scalar1=float(VARF), scalar2=None,
                                        op0=ALU.mult)
                nc.vector.scalar_tensor_tensor(
                    out=rc1[:, 2 * b + 1:2 * b + 2], in0=mv[:, 0:1],
                    scalar=-1.0, op0=ALU.mult,
                    in1=rc1[:, 2 * b:2 * b + 1], op1=ALU.mult)
                rc1h = sb.tile([128, 2], BF16, tag=f"rc1h{b}", bufs=1,
                               name=f"rc1h{b}")
                nc.vector.tensor_copy(out=rc1h, in_=rc1[:, 2 * b:2 * b + 2])
                nc.gpsimd.dma_start(out=x1bh[ST - 1][b][0:128, 512:514],
                                    in_=rc1h)
                nc.vector.scalar_tensor_tensor(
                    out=x1[:, b, :], in0=x1[:, b, :],
                    scalar=rc1[:, 2 * b:2 * b + 1], op0=ALU.mult,
                    in1=rc1[:, 2 * b + 1:2 * b + 2].to_broadcast((128, S)),
                    op1=ALU.add)

            for b in range(B):
                for t in range(ST):
                    attn1_tile(b, t)
                    if t < ST - 1:
                        nc.gpsimd.collective_compute(
                            "AllGather", ALU.bypass, replica_groups=RG,
                            ins=[x1bh[t][b][:]], outs=[x1fh[t][b][:]])
                stats1_close(b)
                nc.gpsimd.collective_compute(
                    "AllGather", ALU.bypass, replica_groups=RG,
                    ins=[x1bh[ST - 1][b][:]], outs=[x1fh[ST - 1][b][:]])
                # encoder k2/v2 for this batch: collective-independent PE
                # work that fills this batch's post-attention gap
                for t in range(ST):
                    xs = sb.tile([128, KC, 512], BF16, tag="xs", bufs=4,
                                 name="xs")
                    nc.sync.dma_start(out=xs, in_=encd[:, b, :, ts(t, 512)])
                    vproj(xs, wv2, kT2, vO2, b, t, kw=wk2, kcol=3)

            if DBG:
                nc.sync.dma_start(out=dbg_x1[:], in_=x1)

            # ================= sublayer 2: cross-attention =================
            # per batch: fold that batch's gathered stats1 into scaled wq2 +
            # constant bias, then q2 projection + attention per tile with a
            # per-tile AllToAll of raw x2 token shards.  Batch 1 does tile 3
            # first so its last AllToAll fires well before the FFN needs it.
            wq2s = sb.tile([128, B, KC, DL], BF16, tag="wq2s", bufs=1,
                           name="wq2s")
            cq2 = sb.tile([128, B], F32, tag="cq2", bufs=1, name="cq2")
            st2 = sb.tile([128, B, ST, 6], F32, tag="st2", bufs=1,
                          name="st2")

            def fold2(b):
                rc1gh = sb.tile([128, KC, 2], BF16, tag=f"rc1g{b}", bufs=1,
                                name=f"rc1g{b}")
                nc.sync.dma_start(
                    out=rc1gh,
                    in_=x1fh[ST - 1][b][:].rearrange(
                        "(r p) s -> p r s", p=128)[:, :, 512:514])
                rc1gf = sb.tile([128, KC, 2], F32, tag=f"rc1gf{b}", bufs=1,
                                name=f"rc1gf{b}")
                nc.vector.tensor_copy(out=rc1gf, in_=rc1gh)
                for k in range(KC):
                    nc.vector.tensor_scalar(
                        out=wq2s[:, b, k, :], in0=wq2[:, k, :],
                        scalar1=rc1gf[:, k, 0:1], scalar2=None,
                        op0=ALU.mult)
                p_ = ps.tile([128, 512], F32, tag="a", name="p_")
                for k in range(KC):
                    nc.tensor.matmul(p_[:, 0:1], wq2[:, k, :],
                                     rc1gh[:, k, 1:2],
                                     start=(k == 0), stop=(k == KC - 1))
                nc.vector.tensor_scalar(out=cq2[:, b:b + 1], in0=p_[:, 0:1],
                                        scalar1=bqk_sb[:, 2:3],
                                        scalar2=None, op0=ALU.add)

            def stats2_close(b):
                rc2b = sb.tile([128, 2], F32, tag=f"rc2_{b}", bufs=1,
                               name=f"rc2_{b}")
                mv = sb.tile([128, 2], F32, tag="bnmv", bufs=2, name="mv")
                nc.vector.bn_aggr(out=mv, in_=st2[:, b, :, :])
                nc.vector.reciprocal(rc2b[:, 0:1], mv[:, 1:2])
                nc.vector.tensor_scalar(out=rc2b[:, 0:1], in0=rc2b[:, 0:1],
                                        scalar1=float(VARF), scalar2=None,
                                        op0=ALU.mult)
                nc.vector.scalar_tensor_tensor(
                    out=rc2b[:, 1:2], in0=mv[:, 0:1],
                    scalar=-1.0, op0=ALU.mult,
                    in1=rc2b[:, 0:1], op1=ALU.mult)
                with tc.high_priority():
                    nc.gpsimd.dma_start(out=st2b[b][:], in_=rc2b)
                    nc.gpsimd.collective_compute(
                        "AllGather", ALU.bypass, replica_groups=RG,
                        ins=[st2b[b][:]], outs=[st2f[b][:]])

            def prep2(b, t):
                """q2 projection for one tile (PE + one DVE bias add)."""
                xs = sb.tile([128, KC, 512], BF16, tag="xs", bufs=4,
                             name="xs")
                nc.sync.dma_start(
                    out=xs,
                    in_=x1fh[t][b][:].rearrange(
                        "(r p) s -> p r s", p=128)[:, :, 0:512])
                qt = sb.tile([128, 512], BF16, tag="qt", bufs=3,
                             name="qt")
                p_ = ps.tile([128, 512], F32, tag="a", name="p_")
                for k in range(KC):
                    nc.tensor.matmul(p_, wq2s[:, b, k, :], xs[:, k, :],
                                     start=(k == 0), stop=(k == KC - 1))
                nc.vector.tensor_scalar(out=qt[:, :], in0=p_,
                                        scalar1=cq2[:, b:b + 1],
                                        scalar2=None, op0=ALU.add)
                return qt

            def attn2_chunks(b, t, qt):
                x2st = sb.tile([128, 512], BF16, tag="stage", bufs=2,
                               name="x2st")
                rs1 = sb.tile([64, 512], F32, tag="rs1", bufs=1,
                              name="rs1")
                nc.vector.tensor_copy(out=rs1,
                                      in_=x1[64:128, b, ts(t, 512)])

                def res2(h):
                    if h == 0:
                        return x1[0:64, b, ts(t, 512)]
                    return rs1

                def out2(h):
                    return x2st[h * 64:h * 64 + 64, :]

                fin = attn_one(b, t, qt, kT2, vO2, bv_off=HL, causal=False,
                               resid_cb=res2, out_cb=out2)
                return x2st, fin

            # tile-level software pipeline: the NEXT tile's q2 projection is
            # emitted between this tile's chunk loop and its normalize chain,
            # so the DVE->gpsimd->DVE normalize latency never starves the PE
            for b in range(B):
                fold2(b)
                order = [3, 0, 1, 2] if b == 1 else [0, 1, 2, 3]
                qt = prep2(b, order[0])
                for i, t in enumerate(order):
                    x2st, fin = attn2_chunks(b, t, qt)
                    if i + 1 < ST:
                        qt = prep2(b, order[i + 1])
                    fin()
                    nc.vector.bn_stats(out=st2[:, b, t, :], in_=x2st)
                    nc.gpsimd.dma_start(
                        out=a2ain[2 * t + b][:].rearrange(
                            "(j p) u -> p j u", p=128),
                        in_=x2st[:, :].rearrange("p (j u) -> p j u", u=64))
                    if i == ST - 1:
                        stats2_close(b)
                    nc.gpsimd.collective_compute(
                        "AllToAll", ALU.bypass, replica_groups=RG,
                        ins=[a2ain[2 * t + b][:]],
                        outs=[a2aout[2 * t + b][:]])

            # ================= sublayer 3: token-parallel FFN ==============
            # gather own 512 tokens (columns ordered b-major: b*256 + t*64+u)
            # per batch: gather + normalize + h-phase, so batch-0's h
            # matmuls run while batch-1's last AllToAll + stats are landing
            xg = sb.tile([128, KC, TOK], BF16, tag="xs", bufs=4, name="xg")
            xn = sb.tile([128, KC, TOK], BF16, tag="xn", bufs=1, name="xn")
            hT = sb.tile([128, FC, TOK], BF16, tag="hT", bufs=1, name="hT")
            x3 = sb.tile([128, KC, TOK], F32, tag="x3", bufs=1, name="x3")
            st3 = sb.tile([128, KC, B, 6], F32, tag="st3", bufs=1,
                          name="st3")
            st3g = sb.tile([128, NCORES, KC, B, 6], F32, tag="st3g",
                           bufs=1, name="st3g")
            rc2g = [sb.tile([128, KC, 2], F32, tag=f"rc2g{b}", bufs=1,
                            name=f"rc2g{b}") for b in range(B)]
            for b in range(B):
                for t in range(ST):
                    g = 2 * t + b
                    nc.sync.dma_start(
                        out=xg[:, :, b * 256 + t * 64:b * 256 + t * 64 + 64],
                        in_=a2aout[g][:].rearrange("(r p) u -> p r u",
                                                   p=128))
                nc.sync.dma_start(
                    out=rc2g[b],
                    in_=st2f[b][:].rearrange("(r p) c -> p r c", p=128))
                for k in range(KC):
                    nc.vector.scalar_tensor_tensor(
                        out=xn[:, k, ts(b, 256)], in0=xg[:, k, ts(b, 256)],
                        scalar=rc2g[b][:, k, 0:1], op0=ALU.mult,
                        in1=rc2g[b][:, k, 1:2].to_broadcast((128, 256)),
                        op1=ALU.add)
                for fc in range(FC):
                    w1f = sb.tile([128, KC, 128], BF16, tag="w1f", bufs=3,
                                  name="w1f")
                    nc.scalar.dma_start(out=w1f, in_=w1d[:, fc, :, :])
                    p_ = ps.tile([128, 512], F32, tag="a", name="p_")
                    for k in range(KC):
                        nc.tensor.matmul(p_[:, 0:256], w1f[:, k, :],
                                         xn[:, k, ts(b, 256)],
                                         start=(k == 0), stop=(k == KC - 1))
                    nc.scalar.activation(out=hT[:, fc, ts(b, 256)],
                                         in_=p_[:, 0:256], func=AF.Relu,
                                         bias=b1_sb[:, fc:fc + 1], scale=1.0)
            if DBG:
                nc.sync.dma_start(out=dbg_xn[:], in_=xn)

            pyA = [ps.tile([128, 512], F32, tag="a", name=f"pyA{i}")
                   for i in range(2)]
            pyZ = [ps.tile([128, 512], F32, tag="z", name=f"pyZ{i}")
                   for i in range(2)]
            pyE = [ps.tile([128, 1024], F32, tag="e", name=f"pyE{i}")
                   for i in range(2)]
            py = [pyA[0], pyA[1], pyZ[0], pyZ[1],
                  pyE[0][:, 0:512], pyE[0][:, 512:1024],
                  pyE[1][:, 0:512], pyE[1][:, 512:1024]]
            for fc in range(FC):
                w2f = sb.tile([128, D], BF16, tag="w2f", bufs=3,
                              name="w2f")
                nc.scalar.dma_start(out=w2f, in_=w2d[:, fc, :])
                for ec in range(KC):
                    nc.tensor.matmul(py[ec],
                                     w2f[:, ts(ec, 128)],
                                     hT[:, fc, :],
                                     start=(fc == 0),
                                     stop=(fc == FC - 1))
            for ec in range(KC):
                nc.vector.scalar_tensor_tensor(
                    out=x3[:, ec, :], in0=py[ec],
                    scalar=b2_sb[:, ec:ec + 1], op0=ALU.add,
                    in1=xn[:, ec, :], op1=ALU.add)
                for b in range(B):
                    nc.vector.bn_stats(out=st3[:, ec, b, :],
                                       in_=x3[:, ec, ts(b, 256)])
            with tc.high_priority():
                nc.gpsimd.dma_start(
                    out=st3b[:].rearrange("p (k b c) -> p k b c",
                                          b=B, c=6),
                    in_=st3)
                nc.gpsimd.collective_compute(
                    "AllGather", ALU.bypass, replica_groups=RG,
                    ins=[st3b[:]], outs=[st3f[:]])
            nc.sync.dma_start(
                out=st3g[:].rearrange("p r k b c -> p r (k b c)"),
                in_=st3f[:].rearrange("(r p) c -> p r c", p=128))

            # cross-core stats aggregation + final normalize + output
            # (aggregate all k first, then batched r/c math, then per-k
            # normalize+store -- fewer tiny DVE ops on the critical tail)
            for b in range(B):
                mv3 = sb.tile([128, KC, 2], F32, tag="mv3", bufs=2,
                              name="mv3")
                for k in range(KC):
                    nc.vector.bn_aggr(out=mv3[:, k, :],
                                      in_=st3g[:, :, k, b, :])
                r3 = sb.tile([128, KC], F32, tag="r3", bufs=2, name="r3")
                nc.vector.reciprocal(r3, mv3[:, :, 1])
                nc.vector.tensor_scalar(out=r3, in0=r3,
                                        scalar1=float(VARF),
                                        scalar2=None, op0=ALU.mult)
                c3 = sb.tile([128, KC], F32, tag="c3", bufs=2, name="c3")
                nc.vector.scalar_tensor_tensor(
                    out=c3, in0=mv3[:, :, 0], scalar=-1.0, op0=ALU.mult,
                    in1=r3, op1=ALU.mult)
                for k in range(KC):
                    nc.vector.scalar_tensor_tensor(
                        out=x3[:, k, ts(b, 256)], in0=x3[:, k, ts(b, 256)],
                        scalar=r3[:, k:k + 1], op0=ALU.mult,
                        in1=c3[:, k:k + 1].to_broadcast((128, 256)),
                        op1=ALU.add)
                    nc.sync.dma_start(out=outT[:, k, ts(b, 256)],
                                      in_=x3[:, k, ts(b, 256)])

    nc.compile()
    return nc


def _get_nc():
    global _CACHED_NC
    if _CACHED_NC is None:
        _CACHED_NC = _build()
    return _CACHED_NC


def _chunked(a, dt):
    """[D, N] -> [128, D//128, N] with [p, c, n] = a[128c+p, n]."""
    d, n = a.shape
    return np.ascontiguousarray(
        a.reshape(d // 128, 128, n).transpose(1, 0, 2)).astype(dt)


def _make_in_maps(decoder_input, encode_input,
                  Wq1, Wk1, Wv1, bq1, bk1, bv1,
                  Wq2, Wk2, Wv2, bq2, bk2, bv2,
                  W1, b1, W2, b2):
    import ml_dtypes
    BF = ml_dtypes.bfloat16
    xT = np.transpose(np.asarray(decoder_input, np.float32), (0, 2, 1))
    eT = np.transpose(np.asarray(encode_input, np.float32), (0, 2, 1))
    # [128, B, KC, S] bf16
    xTd_all = np.ascontiguousarray(
        xT.reshape(B, KC, 128, S).transpose(2, 0, 1, 3)).astype(BF)
    encd_all = np.ascontiguousarray(
        eT.reshape(B, KC, 128, S).transpose(2, 0, 1, 3)).astype(BF)
    # full FFN weights, fc-major
    w1_all = np.ascontiguousarray(
        W1.reshape(KC, 128, FC, 128).transpose(1, 2, 0, 3)).astype(BF)
    w2_all = np.ascontiguousarray(
        W2.reshape(FC, 128, D).transpose(1, 0, 2)).astype(BF)
    b1_all = np.ascontiguousarray(
        b1.reshape(FC, 128).T).astype(np.float32)
    b2_all = np.ascontiguousarray(
        b2.reshape(KC, 128).T).astype(np.float32)
    in_maps = []
    for r in range(NCORES):
        hs = slice(DL * r, DL * (r + 1))
        resd = np.ascontiguousarray(
            xT[:, hs, :].transpose(1, 0, 2)).astype(np.float32)  # [128,B,S]
        bqk_arr = np.stack([bq1[hs], bk1[hs], bq2[hs], bk2[hs]],
                           axis=1).astype(np.float32)      # [128, 4]
        bv_arr = np.concatenate([
            bv1[hs].reshape(HL, DK).T, bv2[hs].reshape(HL, DK).T,
        ], axis=1).astype(np.float32)                      # [64, 4]
        in_maps.append({
            "xTd": xTd_all,
            "encd": encd_all,
            "resd": resd,
            "wq1d": _chunked(np.ascontiguousarray(Wq1[:, hs]), BF),
            "wk1d": _chunked(np.ascontiguousarray(Wk1[:, hs]), BF),
            "wv1d": _chunked(np.ascontiguousarray(Wv1[:, hs]), BF),
            "wq2d": _chunked(np.ascontiguousarray(Wq2[:, hs]), BF),
            "wk2d": _chunked(np.ascontiguousarray(Wk2[:, hs]), BF),
            "wv2d": _chunked(np.ascontiguousarray(Wv2[:, hs]), BF),
            "w1d": w1_all,
            "w2d": w2_all,
            "bqkd": bqk_arr,
            "bvd": bv_arr,
            "b1d": b1_all,
            "b2d": b2_all,
        })
    return in_maps


def kernel(**inputs):
    nc = _get_nc()
    in_maps = _make_in_maps(**{k: np.asarray(v) for k, v in inputs.items()})
    res = run_bass_kernel_spmd(nc, in_maps, core_ids=list(range(NCORES)),
                               trace=False)
    out = np.empty((B, S, D), np.float32)
    for j in range(NCORES):
        o = np.asarray(res.results[j]["outT"], np.float32)  # [128, KC, TOK]
        # col = b*256 + t*64 + u ; token = (b, 512*t + 64*j + u)
        o5 = o.reshape(128, KC, B, ST, 64)
        arr = o5.transpose(2, 3, 4, 1, 0).reshape(B, ST, 64, D)
        for b in range(B):
            for t in range(ST):
                q0 = 512 * t + 64 * j
                out[b, q0:q0 + 64, :] = arr[b, t]
    return out

